# revision 45
# baseline (speedup 1.0000x reference)
"""Trainium2 Bass kernel for conv-projected multi-head attention.

Reference computation (per batch element b of 8):
  q  = conv1x1(x, Wq)                     # [512, 32, 32]
  kv = conv3x3(x, Wkv, pad=1)             # [1024, 32, 32] -> k, v
  per head h (8 heads, d=64): attn = softmax(q k^T / sqrt(d)); o = attn v
  out = conv1x1(gelu(o), Wout) + bout     # [256, 32, 32]

Sharding: data-parallel over batch. Core b computes batch element b
end-to-end; no collectives.

Per-core kernel structure (matmuls bf16 -> fp32 PSUM). The tensor
engine charges a matmul by its output free-size (N), so every stage
streams its smallest dim:
  - x held in SBUF zero-padded to [256, 34, 34]; q/k convs are
    weight-stationary over strided views (N=512 pixel chunks); the v
    conv is x-stationary over contiguous shifted copies, landing v
    already transposed as va[j, h, d|1] (65th column of ones).
  - dots transposed: eT[j, i] = exp(scale * k qT) per head, scale
    folded into the exp; et tiles [128 j, 1024 i] bf16.
  - attn@v FLIPPED: po[i, ic-slot, d|s] = sum_j eT[j,i]^T va[j,h,:],
    streaming N=65 per matmul (half the PE cycles of streaming
    pixels); the ones column makes slot element 64 the softmax
    denominator, landing per-partition. po = 2x[128, 4, 65] f32, each
    inside one PSUM bank.
  - normalize on DVE: reciprocal of the s column + one broadcast
    multiply per half-head, writing ogT[i, ic, h, d] bf16.
  - gelu via the tanh approximation: Tanh shares the ACT table with
    Exp (exp_and_others) so no 1283ns table reloads interleaved with
    the dots exps. Cubic/fixup on DVE, tanh on ACT.
  - ogT transposed back to [hd, pix] with PE transpose-matmuls
    (identity rhs, bf16, N=128), per head-pair and nh-half; the 1x1
    Wout conv is non-accumulating into a shared 2-buf PSUM pool and
    accumulated in SBUF by DVE adds (bias folded into the pair-0 add).

Scheduling: PE instructions are emitted in execution order with
fine-grained interleaving — dots matmuls are woven between conv /
attn@v matmuls so the ACT exp (1.04us per [128,1024] tile, the
second-busiest engine) never gates PE: dots for heads 0-2 ride the
k/v convs, head h+3 rides attn@v h. Pair chains (transpose, gelu,
out-proj) are split into nh-halves to shorten the serial tail.
"""

import os
import sys
from contextlib import ExitStack

import numpy as np

sys.path.insert(0, "/opt/trn_rl_repo")

import ml_dtypes  # noqa: E402
import concourse.bass as bass  # noqa: E402
import concourse.tile as tile  # noqa: E402
from concourse import bacc, mybir  # noqa: E402
from concourse.bass_utils import run_bass_kernel_spmd  # noqa: E402

BF16 = ml_dtypes.bfloat16

B, C, H, W = 8, 256, 32, 32
HEADS, D = 8, 64
INNER = HEADS * D  # 512
N = H * W  # 1024
SCALE = D ** -0.5
HP, WP = H + 2, W + 2  # padded image
NCORES = 8

dt = mybir.dt
FP8 = ml_dtypes.float8_e4m3

# fp8 hi/lo split scales: x is split as fp8(8x) + fp8(8x - fp8(8x)), weights
# at 64x, so every conv PSUM result carries a 512x factor.  q and k keep it
# (folded into the exp scale: qk carries 512^2); v keeps it too (the ones
# column of va is 512 so the softmax denominator matches the numerator).
SX, SW = 8.0, 64.0
CSCL = SX * SW  # 512
SCALE_EXP = SCALE / (CSCL * CSCL)
# crossterm order: (w variant, x variant); x-lo last so the first two
# matmuls of a group only need the hi image tile
CROSS = (("h", "h"), ("l", "h"), ("h", "l"))

# tanh-gelu constants: gelu(x) ~= 0.5x(1+tanh(0.79788456(x+0.044715x^3)))
GELU_C = 0.044715
GELU_S = 0.7978845608028654


def drive(*streams):
    """Round-robin generators by weight: [(gen, quanta_per_turn)].
    Each generator yield = one emitted quantum (a few PE matmuls).
    NOTE: when one stream exhausts, the others run to completion."""
    live = [[g, w] for g, w in streams]
    while live:
        for gw in list(live):
            g, w = gw
            for _ in range(w):
                try:
                    next(g)
                except StopIteration:
                    live.remove(gw)
                    break


def limit(g, n):
    """At most n quanta of g, leaving the rest for a later drive —
    bounds a dependent stream so drive() can't run it ahead of the
    producer stream it is paired with."""
    for _ in range(n):
        try:
            yield next(g)
        except StopIteration:
            return


def emit(tc, ins, out_ap):
    """Emit the per-core kernel. ins: dict name->AP, out_ap: [256, 1024] f32."""
    nc = tc.nc
    ctx = tc._emit_ctx  # ExitStack owned by caller

    consts = ctx.enter_context(tc.tile_pool(name="consts", bufs=1))
    # conv-weight pools released after their conv phase: the queue
    # allocator hands their SBUF to the late et pools (heads 6-7)
    cw2 = tc.tile_pool(name="cw2", bufs=1, side="right")
    cw2_cm = cw2.__enter__()
    cw1 = tc.tile_pool(name="cw1", bufs=1, side="right")
    cw1_cm = cw1.__enter__()

    # weight loads: the q taps (tiny) first so the q conv starts early,
    # the hi image on sync/gpsimd in row-halves, the lo image + lo q tap
    # on the (otherwise idle until ~6us) scalar queue, then the k taps.
    wq8 = {v: cw1_cm.tile([128, 2, 512], dt.float8e4, name=f"wq8{v}")
           for v in "hl"}
    nc.sync.dma_start(wq8["h"].rearrange("p c m -> p (c m)"), ins["wq8h"])
    nc.scalar.dma_start(wq8["l"].rearrange("p c m -> p (c m)"), ins["wq8l"])
    xp8 = {v: cw1_cm.tile([128, 2, HP * WP], dt.float8e4, name=f"xp8{v}")
           for v in "hl"}
    # split by image rows so the first q-conv matmuls (rows 0-17) start
    # as soon as the top half lands; hi before lo (x-lo is the last
    # crossterm of each accumulation group)
    half = 18 * WP
    for v, c2, sl, eng in (("h", 0, slice(0, half), nc.sync),
                           ("h", 1, slice(0, half), nc.gpsimd),
                           ("h", 0, slice(half, HP * WP), nc.sync),
                           ("h", 1, slice(half, HP * WP), nc.gpsimd),
                           ("l", 0, slice(0, HP * WP), nc.scalar),
                           ("l", 1, slice(0, HP * WP), nc.scalar)):
        xp8_v = ins[f"xp8{v}"].rearrange("p (c n) -> p c n", c=2, n=HP * WP)
        eng.dma_start(xp8[v][:, c2, sl], xp8_v[:, c2, sl])
    wk8 = {}
    for v in "hl":
        wk8[v] = cw1_cm.tile([128, 9, 2, 512], dt.float8e4, name=f"wk8{v}")
        wk8_v = ins[f"wk8{v}"].rearrange("p (t c m) -> p t c m",
                                         t=9, c=2, m=512)
        nc.sync.dma_start(wk8[v][:, 0:5], wk8_v[:, 0:5])
        nc.gpsimd.dma_start(wk8[v][:, 5:9], wk8_v[:, 5:9])

    # padded image views: [128, c2, 34, 34] per variant
    xv8 = {v: xp8[v].rearrange("p c (h w) -> p c h w", h=HP, w=WP)
           for v in "hl"}

    # contiguous shifted copies, one per 3x3 tap and variant:
    # xs8[t][v] = [128, 2, 1024].  Only the v conv needs these (its
    # stationary operand streams [K, 2, 128] slices); the q/k convs
    # stream 4-dim strided views of xp8.
    xs8 = [{v: cw2_cm.tile([128, 2, N], dt.float8e4, name=f"xs{t}_{v}")
            for v in "hl"} for t in range(9)]
    for t in range(9):
        ky, kx = t // 3, t % 3
        for i, v in enumerate("hl"):
            xsv = xs8[t][v].rearrange("p c (h w) -> p c h w", h=H, w=W)
            for c2 in range(2):
                eng = nc.sync if (t + i + c2) % 2 == 0 else nc.gpsimd
                eng.dma_start(xsv[:, c2],
                              xv8[v][:, c2, ky: ky + 32, kx: kx + 32])

    wv8 = {}
    for v in "hl":
        wv8[v] = cw2_cm.tile([128, 9, 2, 512], dt.float8e4, name=f"wv8{v}")
        wv8_v = ins[f"wv8{v}"].rearrange("p (t c m) -> p t c m",
                                         t=9, c=2, m=512)
        nc.sync.dma_start(wv8[v][:, 0:5], wv8_v[:, 0:5])
        nc.gpsimd.dma_start(wv8[v][:, 5:9], wv8_v[:, 5:9])
    wo_sb = consts.tile([128, 4, 256], dt.bfloat16, name="wo_sb")
    nc.sync.dma_start(wo_sb, ins["wo"])
    bias_sb = consts.tile([128, 2], dt.float32, name="bias_sb")
    nc.sync.dma_start(bias_sb, ins["bias"])
    ident_sb = consts.tile([128, 128], dt.bfloat16, name="ident_sb")
    nc.gpsimd.dma_start(ident_sb, ins["ident"])

    # persistent conv outputs (bf16, [ch_chunk 128, 1024 pix])
    q_sb = [consts.tile([128, N], dt.bfloat16, name=f"q_sb{m}") for m in range(4)]
    k_sb = [consts.tile([128, N], dt.bfloat16, name=f"k_sb{m}") for m in range(4)]
    # va[jc]: [128 pix, head, 64 v + 1 ones] = v^T augmented
    va_sb = [consts.tile([128, HEADS, D + 1], dt.bfloat16, name=f"va{j}")
             for j in range(8)]
    # per-pair [hd, pix] tiles and scratch
    gt_sb = consts.tile([128, 512], dt.bfloat16, name="gt_sb")
    t_sb = consts.tile([128, 512], dt.bfloat16, name="t_sb")
    # softmax reciprocals: [i-part, head, ic]
    rcp_sb = consts.tile([128, HEADS, 8], dt.float32, name="rcp_sb")
    # output accumulator [c-part, co, pix] f32
    oacc_sb = consts.tile([128, 2, N], dt.float32, name="oacc_sb")

    etpool = ctx.enter_context(tc.tile_pool(name="etp", bufs=46))

    et_tiles = {}   # h -> list of 8 eT tiles
    ogt_tiles = {}  # pair g -> [128, 8 ic, 2 h, 64] bf16
    gg_tiles = {}   # pair g -> [128, 1024] bf16

    def dots_head(h, et_pool_for_jc=None, pse_pool=None, halves=False,
                  jc_range=range(8)):
        """Generator: one quantum = one (jc) column block (2 mm + exp).
        With halves=True the two ic-half psum tiles come from a shared
        [128,512] pool (conv phase: 2 exps per block)."""
        g, p = h // 2, h % 2
        ps, pe_ = 64 * p, 64 * p + 64
        et_tiles.setdefault(h, [])
        for jc in jc_range:
            pool = etpool if et_pool_for_jc is None else et_pool_for_jc(jc)
            et = pool.tile([128, N], dt.bfloat16, name="et", tag="et")
            lhsT = k_sb[g][ps:pe_, jc * 128:(jc + 1) * 128]
            if halves:
                for ic in range(2):
                    psh = pse_pool.tile([128, 512], dt.float32, name="eps",
                                        tag="eps")
                    nc.tensor.matmul(psh, lhsT,
                                     q_sb[g][ps:pe_, ic * 512:(ic + 1) * 512],
                                     start=True, stop=True)
                    nc.scalar.activation(et[:, ic * 512:(ic + 1) * 512], psh,
                                         mybir.ActivationFunctionType.Exp,
                                         scale=SCALE_EXP)
            else:
                pse = pse_pool.tile([128, N], dt.float32, name="eps",
                                    tag="eps")
                for ic in range(2):
                    nc.tensor.matmul(pse[:, ic * 512:(ic + 1) * 512], lhsT,
                                     q_sb[g][ps:pe_, ic * 512:(ic + 1) * 512],
                                     start=True, stop=True)
                nc.scalar.activation(et, pse,
                                     mybir.ActivationFunctionType.Exp,
                                     scale=SCALE_EXP)
            et_tiles[h].append(et)
            yield

    def conv_qk(mi_list, is_q, nh_list=(0, 1)):
        """Generator: one quantum = 3 accumulating DoubleRow matmuls of a
        q/k tile (one tap's crossterms, or the q tap's full group).  The
        q conv borrows the (still idle) dots PSUM tiles for its first
        chunks so none of its 8 rapid-fire half-tiles ever waits on a
        drain."""
        taps = [4] if is_q else list(range(9))
        for mi in mi_list:
            qep = None
            if is_q and mi < 2:
                qep = epool.tile([128, N], dt.float32, name="qps", tag="eps")
            for nh in nh_list:
                if qep is not None:
                    pe = qep[:, nh * 512:(nh + 1) * 512]
                else:
                    pe = cpool.tile([128, 512], dt.float32, name="cps",
                                    tag="cps")
                # crossterm-major: the (h,h) taps need only the hi image
                # + hi weights, so the group starts before wk8l/xp8l land
                seq = [(t, a, b) for a, b in CROSS for t in taps]
                y0 = 16 * nh
                for i, (t, a, b) in enumerate(seq):
                    ky, kx = t // 3, t % 3
                    if is_q:
                        lhsT = wq8[a][:, :, mi * 128:(mi + 1) * 128]
                    else:
                        lhsT = wk8[a][:, t, :, mi * 128:(mi + 1) * 128]
                    rhs = xv8[b][:, :, ky + y0: ky + y0 + 16, kx: kx + 32]
                    nc.tensor.matmul(pe, lhsT, rhs, start=(i == 0),
                                     stop=(i == len(seq) - 1),
                                     perf_mode=mybir.MatmulPerfMode.DoubleRow)
                    if i % 3 == 2:
                        yield
                dest = (q_sb if is_q else k_sb)[mi][:, nh * 512:(nh + 1) * 512]
                if is_q and nh == 1:
                    nc.scalar.activation(dest, pe,
                                         mybir.ActivationFunctionType.Copy)
                else:
                    nc.vector.tensor_copy(dest, pe)
                yield

    def conv_v():
        """Generator: one quantum = 3 accumulating DoubleRow matmuls of a
        v tile (one tap's crossterms)."""
        for jc in range(8):
            pv = cpool.tile([128, 512], dt.float32, name="vps", tag="cps")
            seq = [(t, a, b) for a, b in CROSS for t in range(9)]
            for i, (t, a, b) in enumerate(seq):
                lhsT = xs8[t][b][:, :, jc * 128:(jc + 1) * 128]
                rhs = wv8[a][:, t]
                nc.tensor.matmul(pv, lhsT, rhs, start=(i == 0),
                                 stop=(i == len(seq) - 1),
                                 perf_mode=mybir.MatmulPerfMode.DoubleRow)
                if i % 3 == 2:
                    yield
            # ones column is CSCL so the denominator matches va's 512x scale
            nc.vector.memset(va_sb[jc][:, :, D:D + 1], CSCL)
            nc.vector.tensor_copy(va_sb[jc][:, :, 0:D], pv)
            yield

    def attnv_head(h, popool, halves=(0, 1), free_et=None):
        """Generator: one quantum = one ic slot (8 matmuls, N=65). The
        per-half normalize is emitted inline right after its 4 slots
        complete, freeing that po buffer early."""
        g, hp = h // 2, h % 2
        if hp == 0 and 0 in halves:
            ogt_tiles[g] = ogtpool.tile([128, 8, 2, D], dt.bfloat16,
                                        name="ogt", tag="ogt")
        for half in halves:
            po = popool.tile([128, 4, D + 1], dt.float32, name="po", tag="po")
            for s4 in range(4):
                ic = half * 4 + s4
                for jc in range(8):
                    nc.tensor.matmul(po[:, s4, :],
                                     et_tiles[h][jc][:, ic * 128:(ic + 1) * 128],
                                     va_sb[jc][:, h, :],
                                     start=(jc == 0), stop=(jc == 7))
                yield
            # normalize on DVE: reciprocal of s column, broadcast multiply
            sl = slice(half * 4, half * 4 + 4)
            nc.vector.reciprocal(rcp_sb[:, h, sl], po[:, :, D])
            rb = rcp_sb[:, h, sl].broadcast_to((128, 4, D))
            nc.vector.tensor_mul(ogt_tiles[g][:, sl, hp, :],
                                 po[:, :, 0:D], rb)
        if free_et is None:
            free_et = 1 in halves
        if free_et:
            del et_tiles[h]

    def pair_tail(g, tailpool, nh_list=(0, 1)):
        """Generator: transpose + gelu for head pair g, by nh-half.
        One quantum = one transpose matmul."""
        ogt = ogt_tiles[g]
        if g not in gg_tiles:
            gg_tiles[g] = ggpool.tile([128, N], dt.bfloat16, name="gg",
                                      tag="gg")
        gg = gg_tiles[g]
        for nh in nh_list:
            tp = tailpool.tile([128, 4, 128], dt.bfloat16, name="tp", tag="tl")
            for i4 in range(4):
                ic = nh * 4 + i4
                nc.tensor.transpose(tp[:, i4, :], ogt[:, ic, :, :], ident_sb)
                yield
            sl = slice(nh * 512, (nh + 1) * 512)
            tpf = tp.rearrange("p a b -> p (a b)")
            if g >= 1:
                # all exps are done by now: the exp->gelu ACT table
                # switch is free, so use the real Gelu straight off the
                # transpose PSUM. (Earlier pairs would thrash the table.)
                nc.scalar.activation(gg[:, sl], tpf,
                                     mybir.ActivationFunctionType.Gelu)
            else:
                # tanh-approx gelu, DVE-major (Tanh shares the Exp table)
                x = gt_sb
                t = t_sb
                nc.vector.tensor_copy(x, tpf)
                nc.vector.tensor_mul(t, x, x)
                nc.vector.tensor_scalar(t, t, GELU_C, 1.0,
                                        mybir.AluOpType.mult,
                                        mybir.AluOpType.add)
                nc.vector.tensor_mul(t, x, t)
                nc.scalar.activation(t, t,
                                     mybir.ActivationFunctionType.Tanh,
                                     scale=GELU_S)
                nc.vector.tensor_scalar(t, t, 0.5, 0.5,
                                        mybir.AluOpType.mult,
                                        mybir.AluOpType.add)
                nc.vector.tensor_mul(gg[:, sl], x, t)
            yield

    def outproj_pair(g, nh_list=(0, 1)):
        """Generator: one quantum = one out-proj matmul, accumulating
        across pairs in the persistent pfacc PSUM banks. The last pair
        drains each quadrant in 256-col halves — DVE for co=0, ACT for
        co=1 (GPSIMD cannot touch PSUM) — each half DMA'd as soon as it
        lands so the close-out DMA pipeline starts half a tile early."""
        for nh in nh_list:
            for co in range(2):
                idx = co * 2 + nh
                nc.tensor.matmul(pfacc[idx],
                                 wo_sb[:, g, co * 128:(co + 1) * 128],
                                 gg_tiles[g][:, nh * 512:(nh + 1) * 512],
                                 start=(g == 0), stop=(g == 3))
                yield
                if g == 3:
                    dst = oacc_sb[:, co, nh * 512:(nh + 1) * 512]
                    if co == 0:
                        nc.vector.tensor_scalar_add(dst, pfacc[idx],
                                                    bias_sb[:, co:co + 1])
                    else:
                        nc.scalar.activation(
                            dst, pfacc[idx],
                            mybir.ActivationFunctionType.Identity,
                            bias=bias_sb[:, co:co + 1])
                    qeng = (nc.scalar, nc.sync, nc.sync, nc.gpsimd)[idx]
                    qeng.dma_start(
                        out_ap[co * 128:(co + 1) * 128,
                               nh * 512:(nh + 1) * 512], dst)

    def dots_chain(heads, pse_pool, halves):
        for h in heads:
            pool_fn = et_pools.get(h)
            yield from dots_head(h, pool_fn, pse_pool, halves)

    et_pools = {}
    # PSUM plan (8 banks, one LIFO stack, phase-scoped):
    #   conv phase:  cps 4 + eps 4 ([128,1024] dots tiles, heads 0-6)
    #   attn early:  po 2 + tp 2 + dps 4 (dots head 7)
    #   attn late:   po 2 + tp 2 + pfacc 4 (persistent out-proj acc)
    popool = tailpool = None
    cpool_ctx = tc.tile_pool(name="cps", bufs=4, space="PSUM")
    with cpool_ctx as cpool_cm:
        cpool = cpool_cm
        epool_ctx = tc.tile_pool(name="eps", bufs=2, space="PSUM")
        epool = epool_ctx.__enter__()
        # all q chunks (weights arrive first), then k chunk 0 staged by
        # nh-half: heads 0-1 of the dots stream need only q0 + k0, so
        # dots h0 jc0-3 (nh0 columns) weave into k0-nh1 right after the
        # nh0 drain — the ACT exp stream (which gates the whole back
        # half of the kernel) starts ~4us earlier than a solid k0 would
        # allow.  limit() keeps the dots stream from outrunning the k
        # chunks it reads (drive runs leftovers to completion).
        drive((conv_qk([0, 1, 2, 3], True), 1))
        drive((conv_qk([0], False, nh_list=(0,)), 1))
        kchain = dots_chain([0, 1, 2], epool, False)
        drive((conv_qk([0], False, nh_list=(1,)), 2), (limit(kchain, 4), 1))
        drive((conv_qk([1], False), 3), (limit(kchain, 7), 1))
        drive((conv_qk([2, 3], False), 2), (kchain, 1))
        # k-conv weights + padded x released; late et pools take the room
        cw1.__exit__(None, None, None)
        ogtpool = ctx.enter_context(tc.tile_pool(name="ogtp", bufs=2))
        ggpool = ctx.enter_context(tc.tile_pool(name="ggp", bufs=2))
        et2 = ctx.enter_context(tc.tile_pool(name="etp2", bufs=8))
        et_pools[5] = lambda jc: etpool if jc < 6 else et3
        et_pools[6] = lambda jc: et2 if jc < 7 else et3
        def vchain():
            yield from dots_chain([3, 4], epool, False)
            yield from dots_head(5, et_pools[5], epool, False, range(6))
            yield from dots_head(6, et_pools[6], epool, False, range(7))
        drive((conv_v(), 2), (vchain(), 1))
        cw2.__exit__(None, None, None)
        et3 = ctx.enter_context(tc.tile_pool(name="etp3", bufs=12))
        et_pools[7] = lambda jc: et3
        epool_ctx.__exit__(None, None, None)

    # ---- attention: dots h7 and pair tails woven into attn@v ----
    with tc.tile_pool(name="pop", bufs=2, space="PSUM") as popool, \
         tc.tile_pool(name="tlp", bufs=2, space="PSUM") as tailpool:
        dpool = tc.tile_pool(name="dps", bufs=2, space="PSUM")
        dpool_cm = dpool.__enter__()
        pfacc = None
        pfpool_ctx = None
        def chain67_gen():
            yield from dots_head(5, et_pools[5], dpool_cm, False, range(6, 8))
            yield from dots_head(6, et_pools[6], dpool_cm, False, range(7, 8))
            yield from dots_head(7, et_pools[7], dpool_cm, False)
        chain67 = chain67_gen()
        for h in range(7):
            if h == 2:
                # all dots done: swap the dots PSUM for the out-proj
                # accumulator banks
                dpool.__exit__(None, None, None)
                pfpool_ctx = tc.tile_pool(name="pfa", bufs=4, space="PSUM")
                pfpool = pfpool_ctx.__enter__()
                # one tile per output quadrant: a shared tile would add
                # a false tile-level dep between one quadrant's drain
                # and the next quadrant's accumulating matmul
                pfacc = [pfpool.tile([128, 512], dt.float32,
                                     name=f"pfacc{q}", tag="pfa")
                         for q in range(4)]
            streams = []
            if h < 2:
                streams.append((chain67, 1))
            if h == 5:
                # warm the gelu ACT table while ACT is past its last
                # exp/tanh: the 1283ns load hides here instead of
                # blocking pair 2's Gelu
                nc.scalar.activation(t_sb[0:1, 0:1], t_sb[0:1, 0:1],
                                     mybir.ActivationFunctionType.Gelu)
            if h % 2 == 0 and h >= 2:
                streams.append((pair_tail(h // 2 - 1, tailpool), 1))
            if h == 3:
                streams.append((outproj_pair(0), 1))
            if h == 6:
                streams.append((outproj_pair(1), 1))
            streams.append((attnv_head(h, popool), 3 if h < 2 else 1))
            drive(*streams)
        # h=7: attn@v solid so the tail chain (norm -> transpose ->
        # gelu -> out-proj) starts asap.  half1 runs FIRST: its chain is
        # the long pole (gelu + out-proj + drain + DMA), so kicking it
        # off early lets half0's chain overlap the close-out; outproj2
        # and pair3 fill PE while the chain's DVE/ACT hops run
        drive((attnv_head(7, popool, halves=(1,), free_et=False), 1))
        drive((attnv_head(7, popool, halves=(0,), free_et=True), 1))
        # hold back 3 outproj2 matmuls: they fill the PE bubble while
        # pair3-nh1's gelu runs on ACT
        op2 = outproj_pair(2)
        drive((pair_tail(3, tailpool, nh_list=(1,)), 1),
              (limit(op2, 5), 1))
        drive((pair_tail(3, tailpool, nh_list=(0,)), 4), (op2, 1),
              (outproj_pair(3, nh_list=(1,)), 1))
        drive((outproj_pair(3, nh_list=(0,)), 1))
        pfpool_ctx.__exit__(None, None, None)


def build_nc(repeat=1):
    nc = bacc.Bacc(trn_type="TRN2", target_bir_lowering=False, debug=False)
    ins = {
        "wo": nc.dram_tensor("wo", [128, 4 * 256], dt.bfloat16,
                             kind="ExternalInput").ap(),
        "bias": nc.dram_tensor("bias", [128, 2], dt.float32,
                               kind="ExternalInput").ap(),
        "ident": nc.dram_tensor("ident", [128, 128], dt.bfloat16,
                                kind="ExternalInput").ap(),
    }
    for v in "hl":
        ins[f"xp8{v}"] = nc.dram_tensor(
            f"xp8{v}", [128, 2 * HP * WP], dt.float8e4,
            kind="ExternalInput").ap()
        ins[f"wq8{v}"] = nc.dram_tensor(
            f"wq8{v}", [128, 2 * 512], dt.float8e4,
            kind="ExternalInput").ap()
        ins[f"wk8{v}"] = nc.dram_tensor(
            f"wk8{v}", [128, 9 * 2 * 512], dt.float8e4,
            kind="ExternalInput").ap()
        ins[f"wv8{v}"] = nc.dram_tensor(
            f"wv8{v}", [128, 9 * 2 * 512], dt.float8e4,
            kind="ExternalInput").ap()
    out_ap = nc.dram_tensor("out", [256, N], dt.float32,
                            kind="ExternalOutput").ap()
    with tile.TileContext(nc) as tc:
        for _ in range(repeat):
            with ExitStack() as ctx:
                tc._emit_ctx = ctx
                emit(tc, ins, out_ap)
    nc.compile()
    return nc


def split8(a):
    """f32 array -> (hi, lo) fp8e4 pair with hi + lo ~= a."""
    hi = a.astype(FP8)
    lo = (a - hi.astype(np.float32)).astype(FP8)
    return hi, lo


def pack_weights(Wq, Wkv, Wout, bout):
    """Host-side packing of weights into the DRAM layouts the kernel expects.

    Conv weights are scaled by SW=64 and split into fp8e4 hi/lo pairs.
    Layouts: wq8 [128 cin, 2 c2, 512 cout]; wk8/wv8 [128, 9 t, 2 c2, 512]."""
    out = {}
    q = Wq[:, :, 0, 0].T.astype(np.float32) * SW      # [256, 512]
    qh, ql = split8(q)
    for v, a in (("h", qh), ("l", ql)):
        out[f"wq8{v}"] = np.ascontiguousarray(
            a.reshape(2, 128, 512).transpose(1, 0, 2).reshape(128, 2 * 512))
    for name, sl in (("wk8", slice(0, INNER)), ("wv8", slice(INNER, None))):
        w = np.stack([Wkv[sl, :, t // 3, t % 3].T for t in range(9)])
        w = w.astype(np.float32) * SW                  # [9, 256, 512]
        wh, wl = split8(w)
        for v, a in (("h", wh), ("l", wl)):
            out[f"{name}{v}"] = np.ascontiguousarray(
                a.transpose(1, 0, 2)                   # [256, 9, 512]
                 .reshape(2, 128, 9, 512)
                 .transpose(1, 2, 0, 3)                # [128, 9, 2, 512]
                 .reshape(128, 9 * 2 * 512))
    out["wo"] = (Wout[:, :, 0, 0].T                    # [512, 256]
                 .reshape(4, 128, 256)
                 .transpose(1, 0, 2)
                 .reshape(128, 4 * 256).astype(BF16))
    out["bias"] = np.ascontiguousarray(
        bout.reshape(2, 128).T).astype(np.float32)
    return out


def pack_x(xb):
    """One batch element [256, 32, 32] -> padded, scaled by SX=8, split
    into fp8e4 hi/lo [128, 2*34*34] pairs."""
    xpad = np.zeros((C, HP, WP), np.float32)
    xpad[:, 1:33, 1:33] = xb * SX
    a = np.ascontiguousarray(
        xpad.reshape(2, 128, HP * WP).transpose(1, 0, 2)
            .reshape(128, 2 * HP * WP))
    return split8(a)


_compiled = {}


def kernel(x, Wq, Wkv, Wout, bout, _trace=False, _tmpdir=None):
    x = np.asarray(x, np.float32)
    Wq = np.asarray(Wq, np.float32)
    Wkv = np.asarray(Wkv, np.float32)
    Wout = np.asarray(Wout, np.float32)
    bout = np.asarray(bout, np.float32)

    if "nc" not in _compiled:
        _compiled["nc"] = build_nc()
    nc = _compiled["nc"]

    wmap = pack_weights(Wq, Wkv, Wout, bout)
    wmap["ident"] = np.eye(128, dtype=np.float32).astype(BF16)
    in_maps = []
    for b in range(NCORES):
        xh, xl = pack_x(x[b])
        in_maps.append({"xp8h": xh, "xp8l": xl, **wmap})

    res = run_bass_kernel_spmd(nc, in_maps, core_ids=list(range(NCORES)),
                               trace=_trace, tmpdir=_tmpdir)
    outs = [res.results[b]["out"].reshape(C, H, W) for b in range(NCORES)]
    full = np.stack(outs).astype(np.float32)
    if _trace:
        return full, res
    return full



# revision 48
# speedup vs baseline: 1.0016x; 1.0016x over previous
"""Trainium2 Bass kernel for conv-projected multi-head attention.

Reference computation (per batch element b of 8):
  q  = conv1x1(x, Wq)                     # [512, 32, 32]
  kv = conv3x3(x, Wkv, pad=1)             # [1024, 32, 32] -> k, v
  per head h (8 heads, d=64): attn = softmax(q k^T / sqrt(d)); o = attn v
  out = conv1x1(gelu(o), Wout) + bout     # [256, 32, 32]

Sharding: data-parallel over batch. Core b computes batch element b
end-to-end; no collectives.

Per-core kernel structure (matmuls bf16 -> fp32 PSUM). The tensor
engine charges a matmul by its output free-size (N), so every stage
streams its smallest dim:
  - x held in SBUF zero-padded to [256, 34, 34]; q/k convs are
    weight-stationary over strided views (N=512 pixel chunks); the v
    conv is x-stationary over contiguous shifted copies, landing v
    already transposed as va[j, h, d|1] (65th column of ones).
  - dots transposed: eT[j, i] = exp(scale * k qT) per head, scale
    folded into the exp; et tiles [128 j, 1024 i] bf16.
  - attn@v FLIPPED: po[i, ic-slot, d|s] = sum_j eT[j,i]^T va[j,h,:],
    streaming N=65 per matmul (half the PE cycles of streaming
    pixels); the ones column makes slot element 64 the softmax
    denominator, landing per-partition. po = 2x[128, 4, 65] f32, each
    inside one PSUM bank.
  - normalize on DVE: reciprocal of the s column + one broadcast
    multiply per half-head, writing ogT[i, ic, h, d] bf16.
  - gelu via the tanh approximation: Tanh shares the ACT table with
    Exp (exp_and_others) so no 1283ns table reloads interleaved with
    the dots exps. Cubic/fixup on DVE, tanh on ACT.
  - ogT transposed back to [hd, pix] with PE transpose-matmuls
    (identity rhs, bf16, N=128), per head-pair and nh-half; the 1x1
    Wout conv is non-accumulating into a shared 2-buf PSUM pool and
    accumulated in SBUF by DVE adds (bias folded into the pair-0 add).

Scheduling: PE instructions are emitted in execution order with
fine-grained interleaving — dots matmuls are woven between conv /
attn@v matmuls so the ACT exp (1.04us per [128,1024] tile, the
second-busiest engine) never gates PE: dots for heads 0-2 ride the
k/v convs, head h+3 rides attn@v h. Pair chains (transpose, gelu,
out-proj) are split into nh-halves to shorten the serial tail.
"""

import os
import sys
from contextlib import ExitStack

import numpy as np

sys.path.insert(0, "/opt/trn_rl_repo")

import ml_dtypes  # noqa: E402
import concourse.bass as bass  # noqa: E402
import concourse.tile as tile  # noqa: E402
from concourse import bacc, mybir  # noqa: E402
from concourse.bass_utils import run_bass_kernel_spmd  # noqa: E402

BF16 = ml_dtypes.bfloat16

B, C, H, W = 8, 256, 32, 32
HEADS, D = 8, 64
INNER = HEADS * D  # 512
N = H * W  # 1024
SCALE = D ** -0.5
HP, WP = H + 2, W + 2  # padded image
NCORES = 8

dt = mybir.dt
FP8 = ml_dtypes.float8_e4m3

# fp8 hi/lo split scales: x is split as fp8(8x) + fp8(8x - fp8(8x)), weights
# at 64x, so every conv PSUM result carries a 512x factor.  q and k keep it
# (folded into the exp scale: qk carries 512^2); v keeps it too (the ones
# column of va is 512 so the softmax denominator matches the numerator).
SX, SW = 8.0, 64.0
CSCL = SX * SW  # 512
SCALE_EXP = SCALE / (CSCL * CSCL)
# crossterm order: (w variant, x variant); x-lo last so the first two
# matmuls of a group only need the hi image tile
CROSS = (("h", "h"), ("l", "h"), ("h", "l"))

# tanh-gelu constants: gelu(x) ~= 0.5x(1+tanh(0.79788456(x+0.044715x^3)))
GELU_C = 0.044715
GELU_S = 0.7978845608028654


def drive(*streams):
    """Round-robin generators by weight: [(gen, quanta_per_turn)].
    Each generator yield = one emitted quantum (a few PE matmuls).
    NOTE: when one stream exhausts, the others run to completion."""
    live = [[g, w] for g, w in streams]
    while live:
        for gw in list(live):
            g, w = gw
            for _ in range(w):
                try:
                    next(g)
                except StopIteration:
                    live.remove(gw)
                    break


def limit(g, n):
    """At most n quanta of g, leaving the rest for a later drive —
    bounds a dependent stream so drive() can't run it ahead of the
    producer stream it is paired with."""
    for _ in range(n):
        try:
            yield next(g)
        except StopIteration:
            return


def emit(tc, ins, out_ap):
    """Emit the per-core kernel. ins: dict name->AP, out_ap: [256, 1024] f32."""
    nc = tc.nc
    ctx = tc._emit_ctx  # ExitStack owned by caller

    consts = ctx.enter_context(tc.tile_pool(name="consts", bufs=1))
    # conv-weight pools released after their conv phase: the queue
    # allocator hands their SBUF to the late et pools (heads 6-7)
    cw2 = tc.tile_pool(name="cw2", bufs=1, side="right")
    cw2_cm = cw2.__enter__()
    cw1 = tc.tile_pool(name="cw1", bufs=1, side="right")
    cw1_cm = cw1.__enter__()

    # weight loads: the q taps (tiny) first so the q conv starts early,
    # the hi image on sync/gpsimd in row-halves, the lo image + lo q tap
    # on the (otherwise idle until ~6us) scalar queue, then the k taps.
    wq8 = {v: cw1_cm.tile([128, 2, 512], dt.float8e4, name=f"wq8{v}")
           for v in "hl"}
    nc.sync.dma_start(wq8["h"].rearrange("p c m -> p (c m)"), ins["wq8h"])
    nc.scalar.dma_start(wq8["l"].rearrange("p c m -> p (c m)"), ins["wq8l"])
    xp8 = {v: cw1_cm.tile([128, 2, HP * WP], dt.float8e4, name=f"xp8{v}")
           for v in "hl"}
    # split by image rows so the first q-conv matmuls (rows 0-17) start
    # as soon as the top half lands; hi before lo (x-lo is the last
    # crossterm of each accumulation group)
    half = 18 * WP
    for v, c2, sl, eng in (("h", 0, slice(0, half), nc.sync),
                           ("h", 1, slice(0, half), nc.gpsimd),
                           ("h", 0, slice(half, HP * WP), nc.sync),
                           ("h", 1, slice(half, HP * WP), nc.gpsimd),
                           ("l", 0, slice(0, HP * WP), nc.scalar),
                           ("l", 1, slice(0, HP * WP), nc.scalar)):
        xp8_v = ins[f"xp8{v}"].rearrange("p (c n) -> p c n", c=2, n=HP * WP)
        eng.dma_start(xp8[v][:, c2, sl], xp8_v[:, c2, sl])
    wk8 = {}
    for v in "hl":
        wk8[v] = cw1_cm.tile([128, 9, 2, 512], dt.float8e4, name=f"wk8{v}")
        wk8_v = ins[f"wk8{v}"].rearrange("p (t c m) -> p t c m",
                                         t=9, c=2, m=512)
        nc.sync.dma_start(wk8[v][:, 0:5], wk8_v[:, 0:5])
        nc.gpsimd.dma_start(wk8[v][:, 5:9], wk8_v[:, 5:9])

    # padded image views: [128, c2, 34, 34] per variant
    xv8 = {v: xp8[v].rearrange("p c (h w) -> p c h w", h=HP, w=WP)
           for v in "hl"}

    # contiguous shifted copies, one per 3x3 tap and variant:
    # xs8[t][v] = [128, 2, 1024].  Only the v conv needs these (its
    # stationary operand streams [K, 2, 128] slices); the q/k convs
    # stream 4-dim strided views of xp8.
    xs8 = [{v: cw2_cm.tile([128, 2, N], dt.float8e4, name=f"xs{t}_{v}")
            for v in "hl"} for t in range(9)]
    for t in range(9):
        ky, kx = t // 3, t % 3
        for i, v in enumerate("hl"):
            xsv = xs8[t][v].rearrange("p c (h w) -> p c h w", h=H, w=W)
            for c2 in range(2):
                eng = nc.sync if (t + i + c2) % 2 == 0 else nc.gpsimd
                eng.dma_start(xsv[:, c2],
                              xv8[v][:, c2, ky: ky + 32, kx: kx + 32])

    wv8 = {}
    for v in "hl":
        wv8[v] = cw2_cm.tile([128, 9, 2, 512], dt.float8e4, name=f"wv8{v}")
        wv8_v = ins[f"wv8{v}"].rearrange("p (t c m) -> p t c m",
                                         t=9, c=2, m=512)
        nc.sync.dma_start(wv8[v][:, 0:5], wv8_v[:, 0:5])
        nc.gpsimd.dma_start(wv8[v][:, 5:9], wv8_v[:, 5:9])
    wo_sb = consts.tile([128, 4, 256], dt.bfloat16, name="wo_sb")
    nc.sync.dma_start(wo_sb, ins["wo"])
    bias_sb = consts.tile([128, 2], dt.float32, name="bias_sb")
    nc.sync.dma_start(bias_sb, ins["bias"])
    ident_sb = consts.tile([128, 128], dt.bfloat16, name="ident_sb")
    nc.gpsimd.dma_start(ident_sb, ins["ident"])

    # persistent conv outputs (bf16, [ch_chunk 128, 1024 pix])
    q_sb = [consts.tile([128, N], dt.bfloat16, name=f"q_sb{m}") for m in range(4)]
    k_sb = [consts.tile([128, N], dt.bfloat16, name=f"k_sb{m}") for m in range(4)]
    # va[jc]: [128 pix, head, 64 v + 1 ones] = v^T augmented
    va_sb = [consts.tile([128, HEADS, D + 1], dt.bfloat16, name=f"va{j}")
             for j in range(8)]
    # per-pair [hd, pix] tiles and scratch
    gt_sb = consts.tile([128, 512], dt.bfloat16, name="gt_sb")
    t_sb = consts.tile([128, 512], dt.bfloat16, name="t_sb")
    # softmax reciprocals: [i-part, head, ic]
    rcp_sb = consts.tile([128, HEADS, 8], dt.float32, name="rcp_sb")
    # output accumulator [c-part, co, pix] f32
    oacc_sb = consts.tile([128, 2, N], dt.float32, name="oacc_sb")

    etpool = ctx.enter_context(tc.tile_pool(name="etp", bufs=46))

    et_tiles = {}   # h -> list of 8 eT tiles
    ogt_tiles = {}  # pair g -> [128, 8 ic, 2 h, 64] bf16
    gg_tiles = {}   # pair g -> [128, 1024] bf16

    def dots_head(h, et_pool_for_jc=None, pse_pool=None, halves=False,
                  jc_range=range(8)):
        """Generator: one quantum = one (jc) column block (2 mm + exp).
        With halves=True the two ic-half psum tiles come from a shared
        [128,512] pool (conv phase: 2 exps per block)."""
        g, p = h // 2, h % 2
        ps, pe_ = 64 * p, 64 * p + 64
        et_tiles.setdefault(h, [])
        for jc in jc_range:
            pool = etpool if et_pool_for_jc is None else et_pool_for_jc(jc)
            et = pool.tile([128, N], dt.bfloat16, name="et", tag="et")
            lhsT = k_sb[g][ps:pe_, jc * 128:(jc + 1) * 128]
            if halves:
                for ic in range(2):
                    psh = pse_pool.tile([128, 512], dt.float32, name="eps",
                                        tag="eps")
                    nc.tensor.matmul(psh, lhsT,
                                     q_sb[g][ps:pe_, ic * 512:(ic + 1) * 512],
                                     start=True, stop=True)
                    nc.scalar.activation(et[:, ic * 512:(ic + 1) * 512], psh,
                                         mybir.ActivationFunctionType.Exp,
                                         scale=SCALE_EXP)
            else:
                pse = pse_pool.tile([128, N], dt.float32, name="eps",
                                    tag="eps")
                for ic in range(2):
                    nc.tensor.matmul(pse[:, ic * 512:(ic + 1) * 512], lhsT,
                                     q_sb[g][ps:pe_, ic * 512:(ic + 1) * 512],
                                     start=True, stop=True)
                nc.scalar.activation(et, pse,
                                     mybir.ActivationFunctionType.Exp,
                                     scale=SCALE_EXP)
            et_tiles[h].append(et)
            yield

    def conv_qk(mi_list, is_q, nh_list=(0, 1)):
        """Generator: one quantum = 3 accumulating DoubleRow matmuls of a
        q/k tile (one tap's crossterms, or the q tap's full group).  The
        q conv borrows the (still idle) dots PSUM tiles for its first
        chunks so none of its 8 rapid-fire half-tiles ever waits on a
        drain."""
        taps = [4] if is_q else list(range(9))
        for mi in mi_list:
            qep = None
            if is_q and mi < 2:
                qep = epool.tile([128, N], dt.float32, name="qps", tag="eps")
            for nh in nh_list:
                if qep is not None:
                    pe = qep[:, nh * 512:(nh + 1) * 512]
                else:
                    pe = cpool.tile([128, 512], dt.float32, name="cps",
                                    tag="cps")
                # crossterm-major: the (h,h) taps need only the hi image
                # + hi weights, so the group starts before wk8l/xp8l land
                seq = [(t, a, b) for a, b in CROSS for t in taps]
                y0 = 16 * nh
                for i, (t, a, b) in enumerate(seq):
                    ky, kx = t // 3, t % 3
                    if is_q:
                        lhsT = wq8[a][:, :, mi * 128:(mi + 1) * 128]
                    else:
                        lhsT = wk8[a][:, t, :, mi * 128:(mi + 1) * 128]
                    rhs = xv8[b][:, :, ky + y0: ky + y0 + 16, kx: kx + 32]
                    nc.tensor.matmul(pe, lhsT, rhs, start=(i == 0),
                                     stop=(i == len(seq) - 1),
                                     perf_mode=mybir.MatmulPerfMode.DoubleRow)
                    if i % 3 == 2:
                        yield
                dest = (q_sb if is_q else k_sb)[mi][:, nh * 512:(nh + 1) * 512]
                if is_q and nh == 1:
                    nc.scalar.activation(dest, pe,
                                         mybir.ActivationFunctionType.Copy)
                else:
                    nc.vector.tensor_copy(dest, pe)
                yield

    def conv_v():
        """Generator: one quantum = 3 accumulating DoubleRow matmuls of a
        v tile (one tap's crossterms)."""
        for jc in range(8):
            pv = cpool.tile([128, 512], dt.float32, name="vps", tag="cps")
            seq = [(t, a, b) for a, b in CROSS for t in range(9)]
            for i, (t, a, b) in enumerate(seq):
                lhsT = xs8[t][b][:, :, jc * 128:(jc + 1) * 128]
                rhs = wv8[a][:, t]
                nc.tensor.matmul(pv, lhsT, rhs, start=(i == 0),
                                 stop=(i == len(seq) - 1),
                                 perf_mode=mybir.MatmulPerfMode.DoubleRow)
                if i % 3 == 2:
                    yield
            # ones column is CSCL so the denominator matches va's 512x scale
            nc.vector.memset(va_sb[jc][:, :, D:D + 1], CSCL)
            nc.vector.tensor_copy(va_sb[jc][:, :, 0:D], pv)
            yield

    def attnv_head(h, popool, halves=(0, 1), free_et=None):
        """Generator: one quantum = one ic slot (8 matmuls, N=65). The
        per-half normalize is emitted inline right after its 4 slots
        complete, freeing that po buffer early."""
        g, hp = h // 2, h % 2
        if hp == 0 and 0 in halves:
            ogt_tiles[g] = ogtpool.tile([128, 8, 2, D], dt.bfloat16,
                                        name="ogt", tag="ogt")
        for half in halves:
            po = popool.tile([128, 4, D + 1], dt.float32, name="po", tag="po")
            for s4 in range(4):
                ic = half * 4 + s4
                for jc in range(8):
                    nc.tensor.matmul(po[:, s4, :],
                                     et_tiles[h][jc][:, ic * 128:(ic + 1) * 128],
                                     va_sb[jc][:, h, :],
                                     start=(jc == 0), stop=(jc == 7))
                yield
            # normalize on DVE: reciprocal of s column, broadcast multiply
            sl = slice(half * 4, half * 4 + 4)
            nc.vector.reciprocal(rcp_sb[:, h, sl], po[:, :, D])
            rb = rcp_sb[:, h, sl].broadcast_to((128, 4, D))
            nc.vector.tensor_mul(ogt_tiles[g][:, sl, hp, :],
                                 po[:, :, 0:D], rb)
        if free_et is None:
            free_et = 1 in halves
        if free_et:
            del et_tiles[h]

    def pair_tail(g, tailpool, nh_list=(0, 1)):
        """Generator: transpose + gelu for head pair g, by nh-half.
        One quantum = one transpose matmul."""
        ogt = ogt_tiles[g]
        if g not in gg_tiles:
            gg_tiles[g] = ggpool.tile([128, N], dt.bfloat16, name="gg",
                                      tag="gg")
        gg = gg_tiles[g]
        for nh in nh_list:
            tp = tailpool.tile([128, 4, 128], dt.bfloat16, name="tp", tag="tl")
            for i4 in range(4):
                ic = nh * 4 + i4
                nc.tensor.transpose(tp[:, i4, :], ogt[:, ic, :, :], ident_sb)
                yield
            sl = slice(nh * 512, (nh + 1) * 512)
            tpf = tp.rearrange("p a b -> p (a b)")
            if g >= 1:
                # all exps are done by now: the exp->gelu ACT table
                # switch is free, so use the real Gelu straight off the
                # transpose PSUM. (Earlier pairs would thrash the table.)
                nc.scalar.activation(gg[:, sl], tpf,
                                     mybir.ActivationFunctionType.Gelu)
            else:
                # tanh-approx gelu, DVE-major (Tanh shares the Exp table)
                x = gt_sb
                t = t_sb
                nc.vector.tensor_copy(x, tpf)
                nc.vector.tensor_mul(t, x, x)
                nc.vector.tensor_scalar(t, t, GELU_C, 1.0,
                                        mybir.AluOpType.mult,
                                        mybir.AluOpType.add)
                nc.vector.tensor_mul(t, x, t)
                nc.scalar.activation(t, t,
                                     mybir.ActivationFunctionType.Tanh,
                                     scale=GELU_S)
                nc.vector.tensor_scalar(t, t, 0.5, 0.5,
                                        mybir.AluOpType.mult,
                                        mybir.AluOpType.add)
                nc.vector.tensor_mul(gg[:, sl], x, t)
            yield

    def outproj_pair(g, nh_list=(0, 1)):
        """Generator: one quantum = one out-proj matmul, accumulating
        across pairs in the persistent pfacc PSUM banks. The last pair
        drains each quadrant in 256-col halves — DVE for co=0, ACT for
        co=1 (GPSIMD cannot touch PSUM) — each half DMA'd as soon as it
        lands so the close-out DMA pipeline starts half a tile early."""
        for nh in nh_list:
            for co in range(2):
                idx = co * 2 + nh
                nc.tensor.matmul(pfacc[idx],
                                 wo_sb[:, g, co * 128:(co + 1) * 128],
                                 gg_tiles[g][:, nh * 512:(nh + 1) * 512],
                                 start=(g == 0), stop=(g == 3))
                yield
                if g == 3:
                    dst = oacc_sb[:, co, nh * 512:(nh + 1) * 512]
                    if co == 0:
                        nc.vector.tensor_scalar_add(dst, pfacc[idx],
                                                    bias_sb[:, co:co + 1])
                    else:
                        nc.scalar.activation(
                            dst, pfacc[idx],
                            mybir.ActivationFunctionType.Identity,
                            bias=bias_sb[:, co:co + 1])
                    qeng = (nc.scalar, nc.sync, nc.sync, nc.gpsimd)[idx]
                    qeng.dma_start(
                        out_ap[co * 128:(co + 1) * 128,
                               nh * 512:(nh + 1) * 512], dst)

    def dots_chain(heads, pse_pool, halves):
        for h in heads:
            pool_fn = et_pools.get(h)
            yield from dots_head(h, pool_fn, pse_pool, halves)

    et_pools = {}
    # PSUM plan (8 banks, one LIFO stack, phase-scoped):
    #   conv phase:  cps 4 + eps 4 ([128,1024] dots tiles, heads 0-6)
    #   attn early:  po 2 + tp 2 + dps 4 (dots head 7)
    #   attn late:   po 2 + tp 2 + pfacc 4 (persistent out-proj acc)
    popool = tailpool = None
    cpool_ctx = tc.tile_pool(name="cps", bufs=4, space="PSUM")
    with cpool_ctx as cpool_cm:
        cpool = cpool_cm
        epool_ctx = tc.tile_pool(name="eps", bufs=2, space="PSUM")
        epool = epool_ctx.__enter__()
        # all q chunks (weights arrive first), then k chunk 0 staged by
        # nh-half: heads 0-1 of the dots stream need only q0 + k0, so
        # dots h0 jc0-3 (nh0 columns) weave into k0-nh1 right after the
        # nh0 drain — the ACT exp stream (which gates the whole back
        # half of the kernel) starts ~4us earlier than a solid k0 would
        # allow.  limit() keeps the dots stream from outrunning the k
        # chunks it reads (drive runs leftovers to completion).
        drive((conv_qk([0, 1, 2, 3], True), 1))
        drive((conv_qk([0], False, nh_list=(0,)), 1))
        kchain = dots_chain([0, 1, 2], epool, False)
        drive((conv_qk([0], False, nh_list=(1,)), 2), (limit(kchain, 4), 1))
        drive((conv_qk([1], False), 3), (limit(kchain, 7), 1))
        drive((conv_qk([2, 3], False), 2), (kchain, 1))
        # k-conv weights + padded x released; late et pools take the room
        cw1.__exit__(None, None, None)
        ogtpool = ctx.enter_context(tc.tile_pool(name="ogtp", bufs=2))
        ggpool = ctx.enter_context(tc.tile_pool(name="ggp", bufs=2))
        et2 = ctx.enter_context(tc.tile_pool(name="etp2", bufs=8))
        et_pools[5] = lambda jc: etpool if jc < 6 else et3
        et_pools[6] = lambda jc: et2 if jc < 7 else et3
        def vchain():
            yield from dots_chain([3, 4], epool, False)
            yield from dots_head(5, et_pools[5], epool, False, range(6))
            yield from dots_head(6, et_pools[6], epool, False, range(7))
        drive((conv_v(), 2), (vchain(), 1))
        cw2.__exit__(None, None, None)
        et3 = ctx.enter_context(tc.tile_pool(name="etp3", bufs=12))
        et_pools[7] = lambda jc: et3
        epool_ctx.__exit__(None, None, None)

    # ---- attention: dots h7 and pair tails woven into attn@v ----
    with tc.tile_pool(name="pop", bufs=2, space="PSUM") as popool, \
         tc.tile_pool(name="tlp", bufs=2, space="PSUM") as tailpool:
        dpool = tc.tile_pool(name="dps", bufs=2, space="PSUM")
        dpool_cm = dpool.__enter__()
        pfacc = None
        pfpool_ctx = None
        def chain67_gen():
            yield from dots_head(5, et_pools[5], dpool_cm, False, range(6, 8))
            yield from dots_head(6, et_pools[6], dpool_cm, False, range(7, 8))
            yield from dots_head(7, et_pools[7], dpool_cm, False)
        chain67 = chain67_gen()
        for h in range(7):
            if h == 2:
                # all dots done: swap the dots PSUM for the out-proj
                # accumulator banks
                dpool.__exit__(None, None, None)
                pfpool_ctx = tc.tile_pool(name="pfa", bufs=4, space="PSUM")
                pfpool = pfpool_ctx.__enter__()
                # one tile per output quadrant: a shared tile would add
                # a false tile-level dep between one quadrant's drain
                # and the next quadrant's accumulating matmul
                pfacc = [pfpool.tile([128, 512], dt.float32,
                                     name=f"pfacc{q}", tag="pfa")
                         for q in range(4)]
            streams = []
            if h < 2:
                streams.append((chain67, 1))
            if h == 5:
                # warm the gelu ACT table while ACT is past its last
                # exp/tanh: the 1283ns load hides here instead of
                # blocking pair 2's Gelu
                nc.scalar.activation(t_sb[0:1, 0:1], t_sb[0:1, 0:1],
                                     mybir.ActivationFunctionType.Gelu)
            if h % 2 == 0 and h >= 2:
                streams.append((pair_tail(h // 2 - 1, tailpool), 1))
            if h == 3:
                streams.append((outproj_pair(0), 1))
            if h == 6:
                # only the nh1 half here: the nh0-half out-proj matmuls
                # of pairs 1-2 are deferred to the tail as PE filler for
                # the gelu-gated gaps (they are not gelu-3 dependent)
                streams.append((outproj_pair(1, nh_list=(1,)), 1))
            streams.append((attnv_head(h, popool), 3 if h < 2 else 1))
            drive(*streams)
        # h=7: attn@v solid so the tail chain (norm -> transpose ->
        # gelu -> out-proj) starts asap.  half1 runs FIRST: its chain is
        # the long pole (gelu + out-proj + drain + DMA), so kicking it
        # off early lets half0's chain overlap the close-out; outproj2
        # and pair3 fill PE while the chain's DVE/ACT hops run
        drive((attnv_head(7, popool, halves=(1,), free_et=False), 1))
        drive((attnv_head(7, popool, halves=(0,), free_et=True), 1))
        # hold back 3 outproj2 matmuls: they fill the PE bubble while
        # pair3-nh1's gelu runs on ACT
        drive((pair_tail(3, tailpool, nh_list=(1,)), 1),
              (outproj_pair(2, nh_list=(1,)), 1))
        drive((pair_tail(3, tailpool, nh_list=(0,)), 1),
              (outproj_pair(3, nh_list=(1,)), 1),
              (outproj_pair(1, nh_list=(0,)), 1))
        drive((outproj_pair(2, nh_list=(0,)), 1))
        drive((outproj_pair(3, nh_list=(0,)), 1))
        pfpool_ctx.__exit__(None, None, None)


def build_nc(repeat=1):
    nc = bacc.Bacc(trn_type="TRN2", target_bir_lowering=False, debug=False)
    ins = {
        "wo": nc.dram_tensor("wo", [128, 4 * 256], dt.bfloat16,
                             kind="ExternalInput").ap(),
        "bias": nc.dram_tensor("bias", [128, 2], dt.float32,
                               kind="ExternalInput").ap(),
        "ident": nc.dram_tensor("ident", [128, 128], dt.bfloat16,
                                kind="ExternalInput").ap(),
    }
    for v in "hl":
        ins[f"xp8{v}"] = nc.dram_tensor(
            f"xp8{v}", [128, 2 * HP * WP], dt.float8e4,
            kind="ExternalInput").ap()
        ins[f"wq8{v}"] = nc.dram_tensor(
            f"wq8{v}", [128, 2 * 512], dt.float8e4,
            kind="ExternalInput").ap()
        ins[f"wk8{v}"] = nc.dram_tensor(
            f"wk8{v}", [128, 9 * 2 * 512], dt.float8e4,
            kind="ExternalInput").ap()
        ins[f"wv8{v}"] = nc.dram_tensor(
            f"wv8{v}", [128, 9 * 2 * 512], dt.float8e4,
            kind="ExternalInput").ap()
    out_ap = nc.dram_tensor("out", [256, N], dt.float32,
                            kind="ExternalOutput").ap()
    with tile.TileContext(nc) as tc:
        for _ in range(repeat):
            with ExitStack() as ctx:
                tc._emit_ctx = ctx
                emit(tc, ins, out_ap)
    nc.compile()
    return nc


def split8(a):
    """f32 array -> (hi, lo) fp8e4 pair with hi + lo ~= a."""
    hi = a.astype(FP8)
    lo = (a - hi.astype(np.float32)).astype(FP8)
    return hi, lo


def pack_weights(Wq, Wkv, Wout, bout):
    """Host-side packing of weights into the DRAM layouts the kernel expects.

    Conv weights are scaled by SW=64 and split into fp8e4 hi/lo pairs.
    Layouts: wq8 [128 cin, 2 c2, 512 cout]; wk8/wv8 [128, 9 t, 2 c2, 512]."""
    out = {}
    q = Wq[:, :, 0, 0].T.astype(np.float32) * SW      # [256, 512]
    qh, ql = split8(q)
    for v, a in (("h", qh), ("l", ql)):
        out[f"wq8{v}"] = np.ascontiguousarray(
            a.reshape(2, 128, 512).transpose(1, 0, 2).reshape(128, 2 * 512))
    for name, sl in (("wk8", slice(0, INNER)), ("wv8", slice(INNER, None))):
        w = np.stack([Wkv[sl, :, t // 3, t % 3].T for t in range(9)])
        w = w.astype(np.float32) * SW                  # [9, 256, 512]
        wh, wl = split8(w)
        for v, a in (("h", wh), ("l", wl)):
            out[f"{name}{v}"] = np.ascontiguousarray(
                a.transpose(1, 0, 2)                   # [256, 9, 512]
                 .reshape(2, 128, 9, 512)
                 .transpose(1, 2, 0, 3)                # [128, 9, 2, 512]
                 .reshape(128, 9 * 2 * 512))
    out["wo"] = (Wout[:, :, 0, 0].T                    # [512, 256]
                 .reshape(4, 128, 256)
                 .transpose(1, 0, 2)
                 .reshape(128, 4 * 256).astype(BF16))
    out["bias"] = np.ascontiguousarray(
        bout.reshape(2, 128).T).astype(np.float32)
    return out


def pack_x(xb):
    """One batch element [256, 32, 32] -> padded, scaled by SX=8, split
    into fp8e4 hi/lo [128, 2*34*34] pairs."""
    xpad = np.zeros((C, HP, WP), np.float32)
    xpad[:, 1:33, 1:33] = xb * SX
    a = np.ascontiguousarray(
        xpad.reshape(2, 128, HP * WP).transpose(1, 0, 2)
            .reshape(128, 2 * HP * WP))
    return split8(a)


_compiled = {}


def kernel(x, Wq, Wkv, Wout, bout, _trace=False, _tmpdir=None):
    x = np.asarray(x, np.float32)
    Wq = np.asarray(Wq, np.float32)
    Wkv = np.asarray(Wkv, np.float32)
    Wout = np.asarray(Wout, np.float32)
    bout = np.asarray(bout, np.float32)

    if "nc" not in _compiled:
        _compiled["nc"] = build_nc()
    nc = _compiled["nc"]

    wmap = pack_weights(Wq, Wkv, Wout, bout)
    wmap["ident"] = np.eye(128, dtype=np.float32).astype(BF16)
    in_maps = []
    for b in range(NCORES):
        xh, xl = pack_x(x[b])
        in_maps.append({"xp8h": xh, "xp8l": xl, **wmap})

    res = run_bass_kernel_spmd(nc, in_maps, core_ids=list(range(NCORES)),
                               trace=_trace, tmpdir=_tmpdir)
    outs = [res.results[b]["out"].reshape(C, H, W) for b in range(NCORES)]
    full = np.stack(outs).astype(np.float32)
    if _trace:
        return full, res
    return full



# revision 58
# speedup vs baseline: 1.0033x; 1.0017x over previous
"""Trainium2 Bass kernel for conv-projected multi-head attention.

Reference computation (per batch element b of 8):
  q  = conv1x1(x, Wq)                     # [512, 32, 32]
  kv = conv3x3(x, Wkv, pad=1)             # [1024, 32, 32] -> k, v
  per head h (8 heads, d=64): attn = softmax(q k^T / sqrt(d)); o = attn v
  out = conv1x1(gelu(o), Wout) + bout     # [256, 32, 32]

Sharding: data-parallel over batch. Core b computes batch element b
end-to-end; no collectives.

Per-core kernel structure (matmuls bf16 -> fp32 PSUM). The tensor
engine charges a matmul by its output free-size (N), so every stage
streams its smallest dim:
  - x held in SBUF zero-padded to [256, 34, 34]; q/k convs are
    weight-stationary over strided views (N=512 pixel chunks); the v
    conv is x-stationary over contiguous shifted copies, landing v
    already transposed as va[j, h, d|1] (65th column of ones).
  - dots transposed: eT[j, i] = exp(scale * k qT) per head, scale
    folded into the exp; et tiles [128 j, 1024 i] bf16.
  - attn@v FLIPPED: po[i, ic-slot, d|s] = sum_j eT[j,i]^T va[j,h,:],
    streaming N=65 per matmul (half the PE cycles of streaming
    pixels); the ones column makes slot element 64 the softmax
    denominator, landing per-partition. po = 2x[128, 4, 65] f32, each
    inside one PSUM bank.
  - normalize on DVE: reciprocal of the s column + one broadcast
    multiply per half-head, writing ogT[i, ic, h, d] bf16.
  - gelu via the tanh approximation: Tanh shares the ACT table with
    Exp (exp_and_others) so no 1283ns table reloads interleaved with
    the dots exps. Cubic/fixup on DVE, tanh on ACT.
  - ogT transposed back to [hd, pix] with PE transpose-matmuls
    (identity rhs, bf16, N=128), per head-pair and nh-half; the 1x1
    Wout conv is non-accumulating into a shared 2-buf PSUM pool and
    accumulated in SBUF by DVE adds (bias folded into the pair-0 add).

Scheduling: PE instructions are emitted in execution order with
fine-grained interleaving — dots matmuls are woven between conv /
attn@v matmuls so the ACT exp (1.04us per [128,1024] tile, the
second-busiest engine) never gates PE: dots for heads 0-2 ride the
k/v convs, head h+3 rides attn@v h. Pair chains (transpose, gelu,
out-proj) are split into nh-halves to shorten the serial tail.
"""

import os
import sys
from contextlib import ExitStack

import numpy as np

sys.path.insert(0, "/opt/trn_rl_repo")

import ml_dtypes  # noqa: E402
import concourse.bass as bass  # noqa: E402
import concourse.tile as tile  # noqa: E402
from concourse import bacc, mybir  # noqa: E402
from concourse.bass_utils import run_bass_kernel_spmd  # noqa: E402

BF16 = ml_dtypes.bfloat16

B, C, H, W = 8, 256, 32, 32
HEADS, D = 8, 64
INNER = HEADS * D  # 512
N = H * W  # 1024
SCALE = D ** -0.5
HP, WP = H + 2, W + 2  # padded image
NCORES = 8

dt = mybir.dt
FP8 = ml_dtypes.float8_e4m3

# fp8 hi/lo split scales: x is split as fp8(8x) + fp8(8x - fp8(8x)), weights
# at 64x, so every conv PSUM result carries a 512x factor.  q and k keep it
# (folded into the exp scale: qk carries 512^2); v keeps it too (the ones
# column of va is 512 so the softmax denominator matches the numerator).
SX, SW = 8.0, 64.0
CSCL = SX * SW  # 512
SCALE_EXP = SCALE / (CSCL * CSCL)
# crossterm order: (w variant, x variant); x-lo last so the first two
# matmuls of a group only need the hi image tile
CROSS = (("h", "h"), ("l", "h"), ("h", "l"))

# tanh-gelu constants: gelu(x) ~= 0.5x(1+tanh(0.79788456(x+0.044715x^3)))
GELU_C = 0.044715
GELU_S = 0.7978845608028654


def drive(*streams):
    """Round-robin generators by weight: [(gen, quanta_per_turn)].
    Each generator yield = one emitted quantum (a few PE matmuls).
    NOTE: when one stream exhausts, the others run to completion."""
    live = [[g, w] for g, w in streams]
    while live:
        for gw in list(live):
            g, w = gw
            for _ in range(w):
                try:
                    next(g)
                except StopIteration:
                    live.remove(gw)
                    break


def limit(g, n):
    """At most n quanta of g, leaving the rest for a later drive —
    bounds a dependent stream so drive() can't run it ahead of the
    producer stream it is paired with."""
    for _ in range(n):
        try:
            yield next(g)
        except StopIteration:
            return


def emit(tc, ins, out_ap):
    """Emit the per-core kernel. ins: dict name->AP, out_ap: [256, 1024] f32."""
    nc = tc.nc
    ctx = tc._emit_ctx  # ExitStack owned by caller

    consts = ctx.enter_context(tc.tile_pool(name="consts", bufs=1))
    # conv-weight pools released after their conv phase: the queue
    # allocator hands their SBUF to the late et pools (heads 6-7)
    cw2 = tc.tile_pool(name="cw2", bufs=1, side="right")
    cw2_cm = cw2.__enter__()
    cw1 = tc.tile_pool(name="cw1", bufs=1, side="right")
    cw1_cm = cw1.__enter__()

    # weight loads: the q taps (tiny) first so the q conv starts early,
    # the hi image on sync/gpsimd in row-halves, the lo image + lo q tap
    # on the (otherwise idle until ~6us) scalar queue, then the k taps.
    wq8 = {v: cw1_cm.tile([128, 2, 512], dt.float8e4, name=f"wq8{v}")
           for v in "hl"}
    nc.sync.dma_start(wq8["h"].rearrange("p c m -> p (c m)"), ins["wq8h"])
    nc.scalar.dma_start(wq8["l"].rearrange("p c m -> p (c m)"), ins["wq8l"])
    xp8 = {v: cw1_cm.tile([128, 2, HP * WP], dt.float8e4, name=f"xp8{v}")
           for v in "hl"}
    # split by image rows so the first q-conv matmuls (rows 0-17) start
    # as soon as the top half lands; hi before lo (x-lo is the last
    # crossterm of each accumulation group)
    half = 18 * WP
    for v, c2, sl, eng in (("h", 0, slice(0, half), nc.sync),
                           ("h", 1, slice(0, half), nc.gpsimd),
                           ("h", 0, slice(half, HP * WP), nc.sync),
                           ("h", 1, slice(half, HP * WP), nc.gpsimd),
                           ("l", 0, slice(0, HP * WP), nc.scalar),
                           ("l", 1, slice(0, HP * WP), nc.scalar)):
        xp8_v = ins[f"xp8{v}"].rearrange("p (c n) -> p c n", c=2, n=HP * WP)
        eng.dma_start(xp8[v][:, c2, sl], xp8_v[:, c2, sl])
    wk8 = {}
    for v in "hl":
        wk8[v] = cw1_cm.tile([128, 9, 2, 512], dt.float8e4, name=f"wk8{v}")
        wk8_v = ins[f"wk8{v}"].rearrange("p (t c m) -> p t c m",
                                         t=9, c=2, m=512)
        nc.sync.dma_start(wk8[v][:, 0:5], wk8_v[:, 0:5])
        nc.gpsimd.dma_start(wk8[v][:, 5:9], wk8_v[:, 5:9])

    # padded image views: [128, c2, 34, 34] per variant
    xv8 = {v: xp8[v].rearrange("p c (h w) -> p c h w", h=HP, w=WP)
           for v in "hl"}

    # contiguous shifted copies, one per 3x3 tap and variant:
    # xs8[t][v] = [128, 2, 1024].  Only the v conv needs these (its
    # stationary operand streams [K, 2, 128] slices); the q/k convs
    # stream 4-dim strided views of xp8.
    xs8 = [{v: cw2_cm.tile([128, 2, N], dt.float8e4, name=f"xs{t}_{v}")
            for v in "hl"} for t in range(9)]
    for t in range(9):
        ky, kx = t // 3, t % 3
        for i, v in enumerate("hl"):
            xsv = xs8[t][v].rearrange("p c (h w) -> p c h w", h=H, w=W)
            for c2 in range(2):
                eng = nc.sync if (t + i + c2) % 2 == 0 else nc.gpsimd
                eng.dma_start(xsv[:, c2],
                              xv8[v][:, c2, ky: ky + 32, kx: kx + 32])

    wv8 = {}
    for v in "hl":
        wv8[v] = cw2_cm.tile([128, 9, 2, 512], dt.float8e4, name=f"wv8{v}")
        wv8_v = ins[f"wv8{v}"].rearrange("p (t c m) -> p t c m",
                                         t=9, c=2, m=512)
        nc.sync.dma_start(wv8[v][:, 0:5], wv8_v[:, 0:5])
        nc.gpsimd.dma_start(wv8[v][:, 5:9], wv8_v[:, 5:9])
    wo_sb = consts.tile([128, 4, 256], dt.bfloat16, name="wo_sb")
    nc.sync.dma_start(wo_sb, ins["wo"])
    bias_sb = consts.tile([128, 2], dt.float32, name="bias_sb")
    nc.sync.dma_start(bias_sb, ins["bias"])
    ident_sb = consts.tile([128, 128], dt.bfloat16, name="ident_sb")
    nc.gpsimd.dma_start(ident_sb, ins["ident"])

    # persistent conv outputs (bf16, [ch_chunk 128, 1024 pix])
    q_sb = [consts.tile([128, N], dt.bfloat16, name=f"q_sb{m}") for m in range(4)]
    k_sb = [consts.tile([128, N], dt.bfloat16, name=f"k_sb{m}") for m in range(4)]
    # va[jc]: [128 pix, head, 64 v + 1 ones] = v^T augmented
    va_sb = [consts.tile([128, HEADS, D + 1], dt.bfloat16, name=f"va{j}")
             for j in range(8)]
    # per-pair [hd, pix] tiles and scratch
    gt_sb = consts.tile([128, 512], dt.bfloat16, name="gt_sb")
    t_sb = consts.tile([128, 512], dt.bfloat16, name="t_sb")
    # softmax reciprocals: [i-part, head, ic]
    rcp_sb = consts.tile([128, HEADS, 8], dt.float32, name="rcp_sb")
    # output accumulator [c-part, co, pix] f32
    oacc_sb = consts.tile([128, 2, N], dt.float32, name="oacc_sb")

    etpool = ctx.enter_context(tc.tile_pool(name="etp", bufs=46))

    et_tiles = {}   # h -> list of 8 eT tiles
    ogt_tiles = {}  # pair g -> [128, 8 ic, 2 h, 64] bf16
    gg_tiles = {}   # pair g -> [128, 1024] bf16

    def dots_head(h, et_pool_for_jc=None, pse_pool=None, halves=False,
                  jc_range=range(8)):
        """Generator: one quantum = one (jc) column block (2 mm + exp).
        With halves=True the two ic-half psum tiles come from a shared
        [128,512] pool (conv phase: 2 exps per block)."""
        g, p = h // 2, h % 2
        ps, pe_ = 64 * p, 64 * p + 64
        et_tiles.setdefault(h, [])
        for jc in jc_range:
            pool = etpool if et_pool_for_jc is None else et_pool_for_jc(jc)
            et = pool.tile([128, N], dt.bfloat16, name="et", tag="et")
            lhsT = k_sb[g][ps:pe_, jc * 128:(jc + 1) * 128]
            if halves:
                for ic in range(2):
                    psh = pse_pool.tile([128, 512], dt.float32, name="eps",
                                        tag="eps")
                    nc.tensor.matmul(psh, lhsT,
                                     q_sb[g][ps:pe_, ic * 512:(ic + 1) * 512],
                                     start=True, stop=True)
                    nc.scalar.activation(et[:, ic * 512:(ic + 1) * 512], psh,
                                         mybir.ActivationFunctionType.Exp,
                                         scale=SCALE_EXP)
            else:
                pse = pse_pool.tile([128, N], dt.float32, name="eps",
                                    tag="eps")
                for ic in range(2):
                    nc.tensor.matmul(pse[:, ic * 512:(ic + 1) * 512], lhsT,
                                     q_sb[g][ps:pe_, ic * 512:(ic + 1) * 512],
                                     start=True, stop=True)
                nc.scalar.activation(et, pse,
                                     mybir.ActivationFunctionType.Exp,
                                     scale=SCALE_EXP)
            et_tiles[h].append(et)
            yield

    def conv_qk(mi_list, is_q, nh_list=(0, 1)):
        """Generator: one quantum = 3 accumulating DoubleRow matmuls of a
        q/k tile (one tap's crossterms, or the q tap's full group).  The
        q conv borrows the (still idle) dots PSUM tiles for its first
        chunks so none of its 8 rapid-fire half-tiles ever waits on a
        drain."""
        taps = [4] if is_q else list(range(9))
        for mi in mi_list:
            qep = None
            if is_q and mi < 2:
                qep = epool.tile([128, N], dt.float32, name="qps", tag="eps")
            for nh in nh_list:
                if qep is not None:
                    pe = qep[:, nh * 512:(nh + 1) * 512]
                else:
                    pe = cpool.tile([128, 512], dt.float32, name="cps",
                                    tag="cps")
                # crossterm-major: the (h,h) taps need only the hi image
                # + hi weights, so the group starts before wk8l/xp8l land
                seq = [(t, a, b) for a, b in CROSS for t in taps]
                y0 = 16 * nh
                for i, (t, a, b) in enumerate(seq):
                    ky, kx = t // 3, t % 3
                    if is_q:
                        lhsT = wq8[a][:, :, mi * 128:(mi + 1) * 128]
                    else:
                        lhsT = wk8[a][:, t, :, mi * 128:(mi + 1) * 128]
                    rhs = xv8[b][:, :, ky + y0: ky + y0 + 16, kx: kx + 32]
                    nc.tensor.matmul(pe, lhsT, rhs, start=(i == 0),
                                     stop=(i == len(seq) - 1),
                                     perf_mode=mybir.MatmulPerfMode.DoubleRow)
                    if i % 3 == 2:
                        yield
                dest = (q_sb if is_q else k_sb)[mi][:, nh * 512:(nh + 1) * 512]
                if is_q and nh == 1:
                    nc.scalar.activation(dest, pe,
                                         mybir.ActivationFunctionType.Copy)
                else:
                    nc.vector.tensor_copy(dest, pe)
                yield

    def conv_v():
        """Generator: one quantum = 3 accumulating DoubleRow matmuls of a
        v tile (one tap's crossterms)."""
        for jc in range(8):
            pv = cpool.tile([128, 512], dt.float32, name="vps", tag="cps")
            seq = [(t, a, b) for a, b in CROSS for t in range(9)]
            for i, (t, a, b) in enumerate(seq):
                lhsT = xs8[t][b][:, :, jc * 128:(jc + 1) * 128]
                rhs = wv8[a][:, t]
                nc.tensor.matmul(pv, lhsT, rhs, start=(i == 0),
                                 stop=(i == len(seq) - 1),
                                 perf_mode=mybir.MatmulPerfMode.DoubleRow)
                if i % 3 == 2:
                    yield
            # ones column is CSCL so the denominator matches va's 512x scale
            nc.vector.memset(va_sb[jc][:, :, D:D + 1], CSCL)
            nc.vector.tensor_copy(va_sb[jc][:, :, 0:D], pv)
            yield

    def attnv_head(h, popool, halves=(0, 1), free_et=None):
        """Generator: one quantum = one ic slot (8 matmuls, N=65). The
        per-half normalize is emitted inline right after its 4 slots
        complete, freeing that po buffer early."""
        g, hp = h // 2, h % 2
        if hp == 0 and 0 in halves:
            ogt_tiles[g] = ogtpool.tile([128, 8, 2, D], dt.bfloat16,
                                        name="ogt", tag="ogt")
        for half in halves:
            po = popool.tile([128, 4, D + 1], dt.float32, name="po", tag="po")
            for s4 in range(4):
                ic = half * 4 + s4
                for jc in range(8):
                    nc.tensor.matmul(po[:, s4, :],
                                     et_tiles[h][jc][:, ic * 128:(ic + 1) * 128],
                                     va_sb[jc][:, h, :],
                                     start=(jc == 0), stop=(jc == 7))
                # no yield after the last slot: the normalize is emitted
                # in the same quantum so its coarse PE dep stays tight
                if s4 < 3:
                    yield
            # normalize on DVE: reciprocal of s column, broadcast multiply
            sl = slice(half * 4, half * 4 + 4)
            nc.vector.reciprocal(rcp_sb[:, h, sl], po[:, :, D])
            rb = rcp_sb[:, h, sl].broadcast_to((128, 4, D))
            nc.vector.tensor_mul(ogt_tiles[g][:, sl, hp, :],
                                 po[:, :, 0:D], rb)
            yield
        if free_et is None:
            free_et = 1 in halves
        if free_et:
            del et_tiles[h]

    def pair_tail(g, tailpool, nh_list=(0, 1)):
        """Generator: transpose + gelu for head pair g, by nh-half.
        One quantum = one transpose matmul."""
        ogt = ogt_tiles[g]
        if g not in gg_tiles:
            gg_tiles[g] = ggpool.tile([128, N], dt.bfloat16, name="gg",
                                      tag="gg")
        gg = gg_tiles[g]
        for nh in nh_list:
            tp = tailpool.tile([128, 4, 128], dt.bfloat16, name="tp", tag="tl")
            for i4 in range(4):
                ic = nh * 4 + i4
                nc.tensor.transpose(tp[:, i4, :], ogt[:, ic, :, :], ident_sb)
                # no yield after the 4th transpose: the gelu must be
                # emitted before other streams interleave PE work, else
                # Tile's coarse PE-counter dep stalls it on them
                if i4 < 3:
                    yield
            sl = slice(nh * 512, (nh + 1) * 512)
            tpf = tp.rearrange("p a b -> p (a b)")
            if g >= 1:
                # all exps are done by now: the exp->gelu ACT table
                # switch is free, so use the real Gelu straight off the
                # transpose PSUM. (Earlier pairs would thrash the table.)
                nc.scalar.activation(gg[:, sl], tpf,
                                     mybir.ActivationFunctionType.Gelu)
            else:
                # tanh-approx gelu, DVE-major (Tanh shares the Exp table)
                x = gt_sb
                t = t_sb
                nc.vector.tensor_copy(x, tpf)
                nc.vector.tensor_mul(t, x, x)
                nc.vector.tensor_scalar(t, t, GELU_C, 1.0,
                                        mybir.AluOpType.mult,
                                        mybir.AluOpType.add)
                nc.vector.tensor_mul(t, x, t)
                nc.scalar.activation(t, t,
                                     mybir.ActivationFunctionType.Tanh,
                                     scale=GELU_S)
                nc.vector.tensor_scalar(t, t, 0.5, 0.5,
                                        mybir.AluOpType.mult,
                                        mybir.AluOpType.add)
                nc.vector.tensor_mul(gg[:, sl], x, t)
            yield

    def outproj_pair(g, nh_list=(0, 1)):
        """Generator: one quantum = one out-proj matmul, accumulating
        across pairs in the persistent pfacc PSUM banks. The last pair
        drains each quadrant in 256-col halves — DVE for co=0, ACT for
        co=1 (GPSIMD cannot touch PSUM) — each half DMA'd as soon as it
        lands so the close-out DMA pipeline starts half a tile early."""
        for nh in nh_list:
            for co in range(2):
                idx = co * 2 + nh
                nc.tensor.matmul(pfacc[idx],
                                 wo_sb[:, g, co * 128:(co + 1) * 128],
                                 gg_tiles[g][:, nh * 512:(nh + 1) * 512],
                                 start=(g == 0), stop=(g == 3))
                # the quadrant drain is emitted BEFORE yielding: a yield
                # here would let other streams interleave PE work, and
                # Tile's coarse PE-counter dep would then stall the drain
                # on unrelated matmuls emitted after the closing one
                if g == 3:
                    dst = oacc_sb[:, co, nh * 512:(nh + 1) * 512]
                    if co == 0:
                        nc.vector.tensor_scalar_add(dst, pfacc[idx],
                                                    bias_sb[:, co:co + 1])
                    else:
                        nc.scalar.activation(
                            dst, pfacc[idx],
                            mybir.ActivationFunctionType.Identity,
                            bias=bias_sb[:, co:co + 1])
                    qeng = (nc.scalar, nc.sync, nc.sync, nc.gpsimd)[idx]
                    qeng.dma_start(
                        out_ap[co * 128:(co + 1) * 128,
                               nh * 512:(nh + 1) * 512], dst)
                yield

    def dots_chain(heads, pse_pool, halves):
        for h in heads:
            pool_fn = et_pools.get(h)
            yield from dots_head(h, pool_fn, pse_pool, halves)

    et_pools = {}
    # PSUM plan (8 banks, one LIFO stack, phase-scoped):
    #   conv phase:  cps 4 + eps 4 ([128,1024] dots tiles, heads 0-6)
    #   attn early:  po 2 + tp 2 + dps 4 (dots head 7)
    #   attn late:   po 2 + tp 2 + pfacc 4 (persistent out-proj acc)
    popool = tailpool = None
    cpool_ctx = tc.tile_pool(name="cps", bufs=4, space="PSUM")
    with cpool_ctx as cpool_cm:
        cpool = cpool_cm
        epool_ctx = tc.tile_pool(name="eps", bufs=2, space="PSUM")
        epool = epool_ctx.__enter__()
        # all q chunks (weights arrive first), then k chunk 0 staged by
        # nh-half: heads 0-1 of the dots stream need only q0 + k0, so
        # dots h0 jc0-3 (nh0 columns) weave into k0-nh1 right after the
        # nh0 drain — the ACT exp stream (which gates the whole back
        # half of the kernel) starts ~4us earlier than a solid k0 would
        # allow.  limit() keeps the dots stream from outrunning the k
        # chunks it reads (drive runs leftovers to completion).
        drive((conv_qk([0, 1, 2, 3], True), 1))
        drive((conv_qk([0], False, nh_list=(0,)), 1))
        kchain = dots_chain([0, 1, 2], epool, False)
        drive((conv_qk([0], False, nh_list=(1,)), 2), (limit(kchain, 4), 1))
        drive((conv_qk([1], False), 3), (limit(kchain, 7), 1))
        drive((conv_qk([2, 3], False), 2), (kchain, 1))
        # k-conv weights + padded x released; late et pools take the room
        cw1.__exit__(None, None, None)
        ogtpool = ctx.enter_context(tc.tile_pool(name="ogtp", bufs=2))
        ggpool = ctx.enter_context(tc.tile_pool(name="ggp", bufs=3))
        et2 = ctx.enter_context(tc.tile_pool(name="etp2", bufs=7))
        et_pools[5] = lambda jc: etpool if jc < 6 else et3
        et_pools[6] = lambda jc: et2 if jc < 7 else et3
        def vchain():
            yield from dots_chain([3, 4], epool, False)
            yield from dots_head(5, et_pools[5], epool, False, range(6))
            yield from dots_head(6, et_pools[6], epool, False, range(7))
        drive((conv_v(), 2), (vchain(), 1))
        cw2.__exit__(None, None, None)
        et3 = ctx.enter_context(tc.tile_pool(name="etp3", bufs=12))
        et_pools[7] = lambda jc: et3
        epool_ctx.__exit__(None, None, None)

    # ---- attention: dots h7 and pair tails woven into attn@v ----
    with tc.tile_pool(name="pop", bufs=2, space="PSUM") as popool, \
         tc.tile_pool(name="tlp", bufs=2, space="PSUM") as tailpool:
        dpool = tc.tile_pool(name="dps", bufs=2, space="PSUM")
        dpool_cm = dpool.__enter__()
        pfacc = None
        pfpool_ctx = None
        def chain67_gen():
            yield from dots_head(5, et_pools[5], dpool_cm, False, range(6, 8))
            yield from dots_head(6, et_pools[6], dpool_cm, False, range(7, 8))
            yield from dots_head(7, et_pools[7], dpool_cm, False)
        chain67 = chain67_gen()
        for h in range(7):
            if h == 2:
                # all dots done: swap the dots PSUM for the out-proj
                # accumulator banks
                dpool.__exit__(None, None, None)
                pfpool_ctx = tc.tile_pool(name="pfa", bufs=4, space="PSUM")
                pfpool = pfpool_ctx.__enter__()
                # one tile per output quadrant: a shared tile would add
                # a false tile-level dep between one quadrant's drain
                # and the next quadrant's accumulating matmul
                pfacc = [pfpool.tile([128, 512], dt.float32,
                                     name=f"pfacc{q}", tag="pfa")
                         for q in range(4)]
            streams = []
            if h < 2:
                streams.append((chain67, 1))
            if h == 5:
                # warm the gelu ACT table while ACT is past its last
                # exp/tanh: the 1283ns load hides here instead of
                # blocking pair 2's Gelu
                nc.scalar.activation(t_sb[0:1, 0:1], t_sb[0:1, 0:1],
                                     mybir.ActivationFunctionType.Gelu)
            if h % 2 == 0 and h >= 2:
                streams.append((pair_tail(h // 2 - 1, tailpool), 1))
            if h == 3:
                streams.append((outproj_pair(0), 1))
            if h == 6:
                # only the nh1 half here: the nh0-half out-proj matmuls
                # of pairs 1-2 are deferred to the tail as PE filler for
                # the gelu-gated gaps (they are not gelu-3 dependent)
                streams.append((outproj_pair(1, nh_list=(1,)), 1))
            streams.append((attnv_head(h, popool), 3 if h < 2 else 1))
            drive(*streams)
        # h=7: attn@v solid so the tail chain (norm -> transpose ->
        # gelu -> out-proj) starts asap.  half1 runs FIRST: its chain is
        # the long pole (gelu + out-proj + drain + DMA), so kicking it
        # off early lets half0's chain overlap the close-out; outproj2
        # and pair3 fill PE while the chain's DVE/ACT hops run
        drive((attnv_head(7, popool, halves=(1,), free_et=False), 1))
        drive((attnv_head(7, popool, halves=(0,), free_et=True), 1))
        # hold back 3 outproj2 matmuls: they fill the PE bubble while
        # pair3-nh1's gelu runs on ACT
        drive((pair_tail(3, tailpool, nh_list=(1,)), 1),
              (outproj_pair(2, nh_list=(1,)), 1))
        drive((pair_tail(3, tailpool, nh_list=(0,)), 1),
              (outproj_pair(3, nh_list=(1,)), 1),
              (outproj_pair(1, nh_list=(0,)), 1))
        drive((outproj_pair(2, nh_list=(0,)), 1))
        drive((outproj_pair(3, nh_list=(0,)), 1))
        pfpool_ctx.__exit__(None, None, None)


def build_nc(repeat=1):
    nc = bacc.Bacc(trn_type="TRN2", target_bir_lowering=False, debug=False)
    ins = {
        "wo": nc.dram_tensor("wo", [128, 4 * 256], dt.bfloat16,
                             kind="ExternalInput").ap(),
        "bias": nc.dram_tensor("bias", [128, 2], dt.float32,
                               kind="ExternalInput").ap(),
        "ident": nc.dram_tensor("ident", [128, 128], dt.bfloat16,
                                kind="ExternalInput").ap(),
    }
    for v in "hl":
        ins[f"xp8{v}"] = nc.dram_tensor(
            f"xp8{v}", [128, 2 * HP * WP], dt.float8e4,
            kind="ExternalInput").ap()
        ins[f"wq8{v}"] = nc.dram_tensor(
            f"wq8{v}", [128, 2 * 512], dt.float8e4,
            kind="ExternalInput").ap()
        ins[f"wk8{v}"] = nc.dram_tensor(
            f"wk8{v}", [128, 9 * 2 * 512], dt.float8e4,
            kind="ExternalInput").ap()
        ins[f"wv8{v}"] = nc.dram_tensor(
            f"wv8{v}", [128, 9 * 2 * 512], dt.float8e4,
            kind="ExternalInput").ap()
    out_ap = nc.dram_tensor("out", [256, N], dt.float32,
                            kind="ExternalOutput").ap()
    with tile.TileContext(nc) as tc:
        for _ in range(repeat):
            with ExitStack() as ctx:
                tc._emit_ctx = ctx
                emit(tc, ins, out_ap)
    nc.compile()
    return nc


def split8(a):
    """f32 array -> (hi, lo) fp8e4 pair with hi + lo ~= a."""
    hi = a.astype(FP8)
    lo = (a - hi.astype(np.float32)).astype(FP8)
    return hi, lo


def pack_weights(Wq, Wkv, Wout, bout):
    """Host-side packing of weights into the DRAM layouts the kernel expects.

    Conv weights are scaled by SW=64 and split into fp8e4 hi/lo pairs.
    Layouts: wq8 [128 cin, 2 c2, 512 cout]; wk8/wv8 [128, 9 t, 2 c2, 512]."""
    out = {}
    q = Wq[:, :, 0, 0].T.astype(np.float32) * SW      # [256, 512]
    qh, ql = split8(q)
    for v, a in (("h", qh), ("l", ql)):
        out[f"wq8{v}"] = np.ascontiguousarray(
            a.reshape(2, 128, 512).transpose(1, 0, 2).reshape(128, 2 * 512))
    for name, sl in (("wk8", slice(0, INNER)), ("wv8", slice(INNER, None))):
        w = np.stack([Wkv[sl, :, t // 3, t % 3].T for t in range(9)])
        w = w.astype(np.float32) * SW                  # [9, 256, 512]
        wh, wl = split8(w)
        for v, a in (("h", wh), ("l", wl)):
            out[f"{name}{v}"] = np.ascontiguousarray(
                a.transpose(1, 0, 2)                   # [256, 9, 512]
                 .reshape(2, 128, 9, 512)
                 .transpose(1, 2, 0, 3)                # [128, 9, 2, 512]
                 .reshape(128, 9 * 2 * 512))
    out["wo"] = (Wout[:, :, 0, 0].T                    # [512, 256]
                 .reshape(4, 128, 256)
                 .transpose(1, 0, 2)
                 .reshape(128, 4 * 256).astype(BF16))
    out["bias"] = np.ascontiguousarray(
        bout.reshape(2, 128).T).astype(np.float32)
    return out


def pack_x(xb):
    """One batch element [256, 32, 32] -> padded, scaled by SX=8, split
    into fp8e4 hi/lo [128, 2*34*34] pairs."""
    xpad = np.zeros((C, HP, WP), np.float32)
    xpad[:, 1:33, 1:33] = xb * SX
    a = np.ascontiguousarray(
        xpad.reshape(2, 128, HP * WP).transpose(1, 0, 2)
            .reshape(128, 2 * HP * WP))
    return split8(a)


_compiled = {}


def kernel(x, Wq, Wkv, Wout, bout, _trace=False, _tmpdir=None):
    x = np.asarray(x, np.float32)
    Wq = np.asarray(Wq, np.float32)
    Wkv = np.asarray(Wkv, np.float32)
    Wout = np.asarray(Wout, np.float32)
    bout = np.asarray(bout, np.float32)

    if "nc" not in _compiled:
        _compiled["nc"] = build_nc()
    nc = _compiled["nc"]

    wmap = pack_weights(Wq, Wkv, Wout, bout)
    wmap["ident"] = np.eye(128, dtype=np.float32).astype(BF16)
    in_maps = []
    for b in range(NCORES):
        xh, xl = pack_x(x[b])
        in_maps.append({"xp8h": xh, "xp8l": xl, **wmap})

    res = run_bass_kernel_spmd(nc, in_maps, core_ids=list(range(NCORES)),
                               trace=_trace, tmpdir=_tmpdir)
    outs = [res.results[b]["out"].reshape(C, H, W) for b in range(NCORES)]
    full = np.stack(outs).astype(np.float32)
    if _trace:
        return full, res
    return full

